# revision 19
# baseline (speedup 1.0000x reference)
"""Trainium2 Bass kernel for BSplineLayer: y = BSpline(knots, coeffs, k=3)((x - min(x)) / (max(x) - min(x) + 1e-8)).

The reference clips the de Boor interval to [3, 3], so the layer is one cubic
P(xn) evaluated everywhere; normalization folds into composed raw-x
coefficients q_i, evaluated as y = (x^2 + alpha)*(q3*x + q2) + delta.

DVE perf modes measured on this part: fp32 tensor_scalar ~0.5 cyc/elem
(2x_2P), fp16 tensor_scalar ~0.25 (4x), fp16 tensor_tensor ~0.5 (2x_1P);
tensor_reduce / scalar_tensor_tensor always ~1 cyc/elem. The kernel is built
around the fast modes:

- Phase 1: x streams through a 3-deep [128,2048] fp32 staging pool whose ONLY
  consumer is a DVE cast to a kept fp16 copy xh (so input DMA never
  backpressures on ACT); DVE folds fp16 max/min accumulators (TT @2x) as
  pieces land, then two small reduces give the local (max, -min) pair. ACT
  squares xh into a kept fp16 xsq off the critical path. A warm AllReduce
  enqueued at t~8 absorbs the ~43us ncfw first-collective barrier; the real
  8-byte AllReduce(max) of (max, -min) chains right behind it (~11.5us when
  back-to-back on the warm stream; result lands ~100us).
- Phase 2 per [128,4096] chunk: sq' = xsq + alpha (TS @4x, in place),
  t1 = fp16(q3*xh + q2) (TS @4x), u = sq'*t1 (TT @2x, in place) -- 2048
  DVE cyc per 4096 elems/partition; ACT adds delta and upcasts fp16->fp32
  into a rotating output pool in [128,2048] pieces; DMA out runs at ~94% of
  peak over a ~50us window (the post-collective floor). The first chunks are
  sized 1024/1024/2048 so the first store issues ~5us after the coefficient
  chain instead of ~10.

fp16 rounding of xh/xsq/t1/u and the fp16 stats cost ~2e-3 relative error
(vs the 2e-2 budget).
"""

import sys

sys.path.insert(0, "/opt/trn_rl_repo")

import numpy as np

N_CORES = 8
ROWS, COLS = 8192, 4096
R_CORE = ROWS // N_CORES          # 1024 rows per core
P = 128                           # SBUF partitions
N_TILES = R_CORE // P             # 8 tiles of [128, 4096] per core
HALF = 2048                       # load/fold/store granularity
FREE = N_TILES * COLS             # 32768 free elems per partition
N_PIECES = FREE // HALF           # 16
CH4 = 4096                        # phase-2 DVE chunk
DEGREE = 3

_CACHE = {}


def _expand_cubic(knots: np.ndarray, coeffs: np.ndarray) -> np.ndarray:
    """Expand de Boor at interval m=3 into monomial coeffs [a0, a1, a2, a3] (float64)."""
    t = np.asarray(knots, dtype=np.float64)
    c = np.asarray(coeffs, dtype=np.float64)
    k = DEGREE
    m = k  # reference clips searchsorted result to [k, n-1] with n-1 == k
    pm = np.polynomial.polynomial
    d = [np.array([c[m - k + j]], dtype=np.float64) for j in range(k + 1)]
    for r in range(1, k + 1):
        for j in range(k, r - 1, -1):
            tl = t[m - k + j]
            tr = t[m + j + 1 - r]
            inv = 1.0 / (tr - tl)
            alpha = np.array([-tl * inv, inv])
            one_m = np.array([1.0 + tl * inv, -inv])
            d[j] = pm.polyadd(pm.polymul(one_m, d[j - 1]), pm.polymul(alpha, d[j]))
    a = np.zeros(4, dtype=np.float64)
    a[: len(d[k])] = d[k]
    return a


def _build_program():
    import concourse.bass as bass
    import concourse.tile as tile
    from concourse import bacc, bass_isa, mybir

    f32 = mybir.dt.float32
    f16 = mybir.dt.float16
    OP = mybir.AluOpType
    AX = mybir.AxisListType
    AF = mybir.ActivationFunctionType

    nc = bacc.Bacc("TRN2", target_bir_lowering=False, debug=False, num_devices=N_CORES)
    x_ext = nc.declare_dram_parameter("x", [R_CORE, COLS], f32, isOutput=False)
    ac_ext = nc.declare_dram_parameter("ac", [1, 4], f32, isOutput=False)
    y_ext = nc.declare_dram_parameter("y", [R_CORE, COLS], f32, isOutput=True)

    with tile.TileContext(nc) as tc:
        with (
            tc.tile_pool(name="stage", bufs=3) as stage,
            tc.tile_pool(name="keep", bufs=1) as keep,
            tc.tile_pool(name="acc", bufs=1) as accp,
            tc.tile_pool(name="t1p", bufs=1) as t1p,
            tc.tile_pool(name="yp", bufs=4) as yp,
            tc.tile_pool(name="small", bufs=1) as small,
            tc.tile_pool(name="dram", bufs=1, space="DRAM") as dram,
        ):
            # Warm the collective path (ncfw setup + first-collective barrier
            # + core-skew sync) concurrently with phase 1.
            warm_in = dram.tile([1, 2], f32)
            warm_out = dram.tile([1, 2], f32)
            nc.gpsimd.collective_compute(
                "AllReduce", OP.max,
                replica_groups=[list(range(N_CORES))],
                ins=[warm_in[:].opt()], outs=[warm_out[:].opt()],
            )

            # host constants in early (gpsimd is idle until the stats path):
            # ac = [e2a=a2/a3, e1a=a1/a3, a3, a0]
            ac_sb = small.tile([1, 4], f32)
            nc.sync.dma_start(out=ac_sb[:], in_=ac_ext[:])
            AC = small.tile([P, 4], f32)
            nc.gpsimd.partition_broadcast(AC[:], ac_sb[:])
            e2a, e1a, a3c, a0c = (AC[:, i:i + 1] for i in range(4))

            xh = keep.tile([P, FREE], f16, tag="xh")
            xsq = keep.tile([P, FREE], f16, tag="xsq")
            mx = accp.tile([P, HALF], f16, tag="mx")
            mn = accp.tile([P, HALF], f16, tag="mn")

            # ---------------- phase 1: load, cast, fold; square off-path ----
            for p in range(N_PIECES):
                t, h = divmod(p, COLS // HALF)
                xt = stage.tile([P, HALF], f32, tag="xs")
                nc.sync.dma_start(
                    out=xt[:],
                    in_=x_ext[t * P:(t + 1) * P, h * HALF:(h + 1) * HALF])
                sl = slice(p * HALF, (p + 1) * HALF)
                # cast on ACT (Copy f32->f16): keeps DVE free for the folds,
                # so local stats land before the warm collective finishes at
                # any clock
                nc.scalar.copy(xh[:, sl], xt[:])
                if p == 1:
                    nc.vector.tensor_tensor(
                        mx[:], xh[:, 0:HALF], xh[:, HALF:2 * HALF], op=OP.max)
                    nc.vector.tensor_tensor(
                        mn[:], xh[:, 0:HALF], xh[:, HALF:2 * HALF], op=OP.min)
                elif p > 1:
                    nc.vector.tensor_tensor(mx[:], mx[:], xh[:, sl], op=OP.max)
                    nc.vector.tensor_tensor(mn[:], mn[:], xh[:, sl], op=OP.min)
            # squares are not needed until phase 2: emit them after all casts
            # so they fill ACT's idle window under the collective
            for p in range(N_PIECES):
                sl = slice(p * HALF, (p + 1) * HALF)
                nc.scalar.activation(xsq[:, sl], xh[:, sl], AF.Square,
                                     bias=0.0, scale=1.0)

            # narrow the accumulators once so the final reduces halve
            nc.vector.tensor_tensor(mx[:, 0:HALF // 2], mx[:, 0:HALF // 2],
                                    mx[:, HALF // 2:HALF], op=OP.max)
            nc.vector.tensor_tensor(mn[:, 0:HALF // 2], mn[:, 0:HALF // 2],
                                    mn[:, HALF // 2:HALF], op=OP.min)
            pk = small.tile([P, 2], f32)
            nc.vector.tensor_reduce(pk[:, 0:1], mx[:, 0:HALF // 2],
                                    axis=AX.X, op=OP.max)
            rmn = small.tile([P, 1], f32)
            nc.vector.tensor_reduce(rmn[:], mn[:, 0:HALF // 2],
                                    axis=AX.X, op=OP.min)
            nc.vector.tensor_scalar_mul(pk[:, 1:2], rmn[:], -1.0)

            # cross-partition: every partition gets (local_max, -local_min)
            par = small.tile([P, 2], f32)
            nc.gpsimd.partition_all_reduce(par[:], pk[:], channels=P,
                                           reduce_op=bass_isa.ReduceOp.max)

            # cross-core: AllGather the 8 pairs, reduce locally
            cc_in = dram.tile([1, 2], f32)
            cc_out = dram.tile([1, 2], f32)
            nc.sync.dma_start(out=cc_in[:], in_=par[0:1, 0:2])
            nc.gpsimd.collective_compute(
                "AllReduce", OP.max,
                replica_groups=[list(range(N_CORES))],
                ins=[cc_in[:].opt()], outs=[cc_out[:].opt()],
            )
            gg_sb = small.tile([1, 2], f32)
            nc.sync.dma_start(out=gg_sb[:], in_=cc_out[:])
            GG = small.tile([P, 2], f32)
            nc.gpsimd.partition_broadcast(GG[:], gg_sb[:])

            # ------- device scalars: normalization + composed coefficients -------
            # s = 1/(gmax + gnm + eps); b = gnm*s    (gnm = -gmin)
            # y = (xsq + d1)*(q3*x + q2) + delta
            cf = small.tile([P, 6], f32)
            d2c, d1c, q3c, q0c, g1c = (cf[:, i:i + 1] for i in range(5))
            tmp = small.tile([P, 8], f32)
            dd, s_, b_, v, w, v2, p_, de_ = (tmp[:, i:i + 1] for i in range(8))

            nc.vector.scalar_tensor_tensor(dd, GG[:, 0:1], 1e-8, GG[:, 1:2],
                                           op0=OP.add, op1=OP.add)      # range+eps
            nc.vector.reciprocal(s_, dd)
            nc.vector.tensor_tensor(v2, s_, s_, op=OP.mult)             # s^2
            nc.vector.scalar_tensor_tensor(q3c, v2, a3c, s_,
                                           op0=OP.mult, op1=OP.mult)    # q3 = (a3*s^2)*s
            nc.vector.tensor_tensor(b_, GG[:, 1:2], s_, op=OP.mult)     # b = gnm*s
            nc.vector.scalar_tensor_tensor(v, b_, 3.0, e2a,
                                           op0=OP.mult, op1=OP.add)     # 3b+e2a
            nc.vector.scalar_tensor_tensor(g1c, dd, v, q3c,
                                           op0=OP.mult, op1=OP.mult)    # q2 = (v*dd)*q3
            nc.vector.tensor_tensor(w, v, e2a, op=OP.add)               # 3b+2e2a
            nc.vector.scalar_tensor_tensor(w, w, b_, e1a,
                                           op0=OP.mult, op1=OP.add)     # (3b+2e2a)b+e1a
            nc.vector.tensor_tensor(v2, dd, dd, op=OP.mult)             # d^2
            nc.vector.tensor_tensor(d1c, w, v2, op=OP.mult)             # alpha [op 13]

            nc.vector.tensor_tensor(p_, b_, e2a, op=OP.add)             # b+e2a
            nc.vector.scalar_tensor_tensor(p_, p_, b_, e1a,
                                           op0=OP.mult, op1=OP.add)     # (b+e2a)b+e1a
            nc.vector.tensor_tensor(p_, p_, b_, op=OP.mult)
            nc.vector.tensor_tensor(p_, p_, a3c, op=OP.mult)
            nc.vector.tensor_tensor(q0c, p_, a0c, op=OP.add)            # q0
            nc.vector.tensor_tensor(de_, g1c, d1c, op=OP.mult)
            nc.vector.tensor_tensor(de_, q0c, de_, op=OP.subtract)      # delta

            # ACT-owned copy of delta (wait-slot limit workaround)
            actsb = small.tile([P, 1], f32)
            nc.scalar.copy(actsb[:, 0:1], de_)

            # ---------------- phase 2: evaluate + store ----------------
            sizes = ([1024, 1024, 2048] + [CH4] * ((FREE - 2 * CH4) // CH4)
                     + [2048, 1024, 1024])
            assert sum(sizes) == FREE
            off = 0
            piece_i = 0
            for sz in sizes:
                sl4 = slice(off, off + sz)
                t1h = t1p.tile([P, CH4], f16, tag="t1")
                nc.vector.tensor_scalar(t1h[:, 0:sz], xh[:, sl4], q3c, g1c,
                                        op0=OP.mult, op1=OP.add)        # fp16 @4x
                nc.vector.tensor_scalar(xsq[:, sl4], xsq[:, sl4], d1c, None,
                                        op0=OP.add)                     # fp16 @4x
                nc.vector.tensor_tensor(xsq[:, sl4], xsq[:, sl4], t1h[:, 0:sz],
                                        op=OP.mult)                     # fp16 @2x
                # store in pieces matching the chunk size (<= HALF each);
                # a few pieces take the DVE path (TS fp16->fp32 @2x_2P adds
                # the same delta) so neither ACT nor DVE paces the store
                # stream below DMA rate
                st = off
                while st < off + sz:
                    pc = min(HALF, off + sz - st)
                    t, h = divmod(st // HALF, COLS // HALF)
                    cst = st - (st // HALF) * HALF
                    yc = yp.tile([P, HALF], f32, tag="y")
                    if piece_i in (7, 11, 15):
                        nc.vector.tensor_scalar(yc[:, 0:pc],
                                                xsq[:, st:st + pc], de_, None,
                                                op0=OP.add)
                    else:
                        nc.scalar.activation(yc[:, 0:pc], xsq[:, st:st + pc],
                                             AF.Identity, bias=actsb[:, 0:1],
                                             scale=1.0)
                    nc.sync.dma_start(
                        out=y_ext[t * P:(t + 1) * P,
                                  h * HALF + cst:h * HALF + cst + pc],
                        in_=yc[:, 0:pc])
                    piece_i += 1
                    st += pc
                off += sz

    nc.compile()
    return nc


def kernel(x: np.ndarray, knots: np.ndarray, coeffs: np.ndarray) -> np.ndarray:
    from concourse.bass_utils import run_bass_kernel_spmd

    x = np.ascontiguousarray(np.asarray(x, dtype=np.float32))
    assert x.shape == (ROWS, COLS), x.shape

    a = _expand_cubic(knots, coeffs)
    a3 = a[3] if abs(a[3]) > 1e-30 else 1e-30
    ac = np.array([[a[2] / a3, a[1] / a3, a3, a[0]]], dtype=np.float32)

    if "nc" not in _CACHE:
        _CACHE["nc"] = _build_program()
    nc = _CACHE["nc"]

    shards = [x[i * R_CORE:(i + 1) * R_CORE] for i in range(N_CORES)]
    in_maps = [{"x": s, "ac": ac} for s in shards]

    import os
    trace = bool(int(os.environ.get("KERNEL_TRACE", "0")))
    res = run_bass_kernel_spmd(nc, in_maps, core_ids=list(range(N_CORES)),
                               trace=trace)
    if trace and res.exec_time_ns is not None:
        print(f"HW exec time: {res.exec_time_ns} ns")
        _CACHE["last_exec_time_ns"] = res.exec_time_ns
        _CACHE["last_trace"] = res.instructions_and_trace

    out = np.empty((ROWS, COLS), dtype=np.float32)
    for i in range(N_CORES):
        out[i * R_CORE:(i + 1) * R_CORE] = res.results[i]["y"]
    return out
